# revision 39
# baseline (speedup 1.0000x reference)
"""Multi-head attention (B=2, S=4096, E=512, H=8) on 8 Trainium2 NeuronCores.

Sharding: core c handles batch b = c//4 and query rows [qi*1024, (qi+1)*1024)
with qi = c%4. Each core computes K/V projections for its full batch (cheap,
avoids any cross-core reduction), Q projection for its query slice, attention
for all 8 heads over its query rows, and the output projection for its rows.
The host only transposes shards on the way in and concatenates the 8 per-core
[1024, 512] outputs on the way out.

Layout/precision choices (from HW traces):
- All matmul operands are fp16 (same PE rate as bf16, 8x the mantissa);
  x and the weight matrices are rounded to fp16 on the host so the device
  DMAs half the bytes and runs no cast pass. All accumulation is fp32.
- Scores are computed transposed (S^T[k, q]) so the exp'd weights land in
  the [k, q] layout the AV matmul needs as its stationary operand; softmax
  denominators come free from an all-ones column appended to V.
- Q^T is stored zero-padded to 128 partitions per head: a 64-partition
  moving operand streams at half rate on the PE, and the zero half lets the
  two heads of a 128-row m-tile share one stationary K^T load.
- Max-subtraction is skipped: raw scores stay well inside fp32/fp16 range
  here, and softmax without the shift is mathematically identical.
- The ACT engine (exp over 33.5M scores/core, ~1.05us per [128,1024] tile)
  is the bottleneck; emission order keeps it fed from ~35us onward.
"""

from contextlib import ExitStack

import numpy as np

import concourse.bass as bass
import concourse.tile as tile
from concourse import bacc, masks, mybir
from concourse.bass_utils import run_bass_kernel_spmd

F32 = mybir.dt.float32
F16 = mybir.dt.float16
AF = mybir.ActivationFunctionType

B = 2
S = 4096  # keys per batch
SQ = 1024  # query rows per core
E = 512
H = 8
DH = 64
P = 128
NE = E // P  # 4 chunks of the contraction dim E
NM = E // P  # 4 m-tiles (head-pairs)
KB = S // P  # 32 key blocks
QB = SQ // P  # 8 query blocks
SC = S // 512  # 8 key 512-chunks
QC = SQ // 512  # 2 query 512-chunks
SWL = 4  # key blocks per attention sweep
NSW = KB // SWL
SCALE = 1.0 / np.sqrt(DH)


def emit(ctx: ExitStack, tc: tile.TileContext, io: dict):
    nc = tc.nc
    xT = io["xT_kv"]  # [E, S] f32
    xTq = io["xT_q"]  # [E, SQ] f32
    y = io["y"]  # [SQ, E] f32

    const = ctx.enter_context(tc.tile_pool(name="const", bufs=1))
    big = ctx.enter_context(tc.tile_pool(name="big", bufs=1))
    apool = ctx.enter_context(tc.tile_pool(name="apool", bufs=2))
    npool = ctx.enter_context(tc.tile_pool(name="npool", bufs=2))
    rpool = ctx.enter_context(tc.tile_pool(name="rpool", bufs=2))
    opool = ctx.enter_context(tc.tile_pool(name="opool", bufs=H))
    # PSUM: 8 banks. Scores rotate through a 6-bank fp16 tensor in aligned
    # (even, odd) head-pairs so exp reads [128, 2048]; projections, AV
    # accumulators, transposes and the output projection share two rotating
    # 1-bank slots (tag "po").
    ps_att = ctx.enter_context(tc.tile_pool(name="ps_att", bufs=2, space="PSUM"))
    po = ctx.enter_context(tc.tile_pool(name="po", bufs=2, space="PSUM"))
    po_proj = ctx.enter_context(tc.tile_pool(name="po_proj", bufs=2, space="PSUM"))

    # --- constants -------------------------------------------------------
    ident = const.tile([P, P], F16)
    masks.make_identity(nc, ident[:])

    w16 = {}

    def load_w(name):
        wt = const.tile([P, NE, E], F16, tag=f"w_{name}", name=f"w_{name}")
        for ec in range(NE):
            eng = nc.sync if ec % 2 == 0 else nc.gpsimd
            eng.dma_start(wt[:, ec, :], io[name][ec * P : (ec + 1) * P, :])
        w16[name] = wt

    load_w("Wq")

    bq_t = const.tile([P, NM], F32, tag="bq")
    bk_t = const.tile([P, NM], F32, tag="bk")
    for m in range(NM):
        nc.gpsimd.dma_start(bq_t[:, m : m + 1], io["bq"][m * P : (m + 1) * P])
        nc.gpsimd.dma_start(bk_t[:, m : m + 1], io["bk"][m * P : (m + 1) * P])

    def bcast(name):
        t = const.tile([P, E], F32, tag=f"bcast_{name}", name=f"bc_{name}")
        src = io[name]
        ap = bass.AP(tensor=src.tensor, offset=src.offset, ap=[[0, P]] + list(src.ap))
        nc.gpsimd.dma_start(t[:], ap)
        return t

    # --- x^T loads (quarter-major so projections can start early) --------
    xT16 = big.tile([P, NE, S], F16, tag="xT")
    xTq16 = big.tile([P, NE, SQ], F16, tag="xTq")
    XQ = 1024  # x columns per load quarter

    for ec in range(NE):
        eng = nc.sync if ec % 2 == 0 else nc.gpsimd
        eng.dma_start(xTq16[:, ec, :], xTq[ec * P : (ec + 1) * P, :])

    def load_x_quarter(qtr):
        for ec in range(NE):
            eng = nc.sync if ec % 2 == 0 else nc.gpsimd
            eng.dma_start(
                xT16[:, ec, qtr * XQ : (qtr + 1) * XQ],
                xT[ec * P : (ec + 1) * P, qtr * XQ : (qtr + 1) * XQ],
            )

    # --- projection targets ----------------------------------------------
    kT16 = big.tile([P, NM, S], F16, tag="kT")
    # Q^T padded: head h lives in [:, h, :] rows (h%2)*64..+64, rest zero.
    qTp = big.tile([P, H, SQ], F16, tag="qTp")
    nc.gpsimd.memset(qTp[:], 0.0)
    v16 = big.tile([P, KB, H, DH + 1], F16, tag="v")
    nc.gpsimd.memset(v16[:, :, :, DH : DH + 1], 1.0)

    def proj_k(m, sc0, sc1):
        for sc in range(sc0, sc1):
            pt = po_proj.tile([P, 512], F32, tag="pop", name=f"pk{m}_{sc}")
            for ec in range(NE):
                nc.tensor.matmul(
                    pt[:],
                    lhsT=w16["Wk"][:, ec, m * P : (m + 1) * P],
                    rhs=xT16[:, ec, sc * 512 : (sc + 1) * 512],
                    start=(ec == 0),
                    stop=(ec == NE - 1),
                )
            nc.vector.tensor_scalar_add(
                kT16[:, m, sc * 512 : (sc + 1) * 512], pt[:], bk_t[:, m : m + 1]
            )

    def proj_q(m):
        for qc in range(QC):
            pt = po_proj.tile([P, 512], F32, tag="pop", name=f"pq{m}_{qc}")
            for ec in range(NE):
                nc.tensor.matmul(
                    pt[:],
                    lhsT=w16["Wq"][:, ec, m * P : (m + 1) * P],
                    rhs=xTq16[:, ec, qc * 512 : (qc + 1) * 512],
                    start=(ec == 0),
                    stop=(ec == NE - 1),
                )
            for hh in range(2):
                h = 2 * m + hh
                r0 = hh * DH
                nc.vector.tensor_scalar_add(
                    qTp[r0 : r0 + DH, h, qc * 512 : (qc + 1) * 512],
                    pt[r0 : r0 + DH, :],
                    bq_t[r0 : r0 + DH, m : m + 1],
                )

    def proj_v(sb0, sb1):
        for sb in range(sb0, sb1):
            pt = po_proj.tile([P, 512], F32, tag="pop", name=f"pv{sb}")
            for ec in range(NE):
                nc.tensor.matmul(
                    pt[:],
                    lhsT=xT16[:, ec, sb * P : (sb + 1) * P],
                    rhs=w16["Wv"][:, ec, :],
                    start=(ec == 0),
                    stop=(ec == NE - 1),
                )
            nc.vector.tensor_add(
                v16[:, sb, :, 0:DH],
                pt[:].rearrange("p (h d) -> p h d", h=H),
                bv_b[:].rearrange("p (h d) -> p h d", h=H),
            )

    # --- attention --------------------------------------------------------
    y_acc_cell = []

    # attnT reuses xTq16's slot (tag "xTq"): allocated after proj_q's last
    # read of xTq16, by which point the slot is free.
    attnT_cell = []
    o_accs = {}
    rot = [0]  # scores_ps slot rotation

    def pair_sweep(m, sw):
        """Scores+exp+AV for head pair (2m, 2m+1), key blocks sw*SWL..+SWL."""
        heads = (2 * m, 2 * m + 1)
        if sw == 0:
            for h in heads:
                o_accs[h] = opool.tile(
                    [P, QB, DH + 1], F32, tag="oacc", name=f"oacc{h}"
                )
        at = apool.tile([P, SWL, 2, SQ], F16, tag="a")
        for i in range(SWL):
            kb = sw * SWL + i
            # one stationary K^T load serves both heads; the zero half of
            # each head's padded Q^T kills the other head's contribution
            for hh in range(2):
                st_ = ps_att.tile([P, SQ], F32, tag="psa", name=f"sc{m}_{sw}_{i}{hh}")
                for qc in range(QC):
                    nc.tensor.matmul(
                        st_[:, qc * 512 : (qc + 1) * 512],
                        lhsT=kT16[:, m, kb * P : (kb + 1) * P],
                        rhs=qTp[:, heads[hh], qc * 512 : (qc + 1) * 512],
                        start=True,
                        stop=True,
                    )
                nc.scalar.activation(
                    at[:, i, hh, :], st_[:], AF.Exp, scale=float(SCALE)
                )
        for hh in range(2):
            h = heads[hh]
            o_acc = o_accs[h]
            for qb in range(QB):
                ot = po.tile([P, DH + 1], F32, tag="po", name=f"ot{h}_{qb}")
                for i in range(SWL):
                    kb = sw * SWL + i
                    nc.tensor.matmul(
                        ot[:],
                        lhsT=at[:, i, hh, qb * P : (qb + 1) * P],
                        rhs=v16[:, kb, h, :],
                        start=(i == 0),
                        stop=(i == SWL - 1),
                    )
                if sw == 0:
                    nc.vector.tensor_copy(o_acc[:, qb, :], ot[:])
                else:
                    nc.vector.tensor_add(o_acc[:, qb, :], o_acc[:, qb, :], ot[:])

    def finish_head(h):
        m, r0 = h // 2, (h % 2) * DH
        o_acc = o_accs[h]
        attnT = attnT_cell[m]
        for qb in range(QB):
            rt = rpool.tile([P, 1], F32, tag="r")
            nc.vector.reciprocal(rt[:], o_acc[:, qb, DH : DH + 1])
            nt = npool.tile([P, DH], F16, tag="n")
            nc.vector.tensor_scalar_mul(nt[:], o_acc[:, qb, 0:DH], rt[:])
            tp = po.tile([DH, P], F16, tag="po", name=f"tp{h}_{qb}")
            nc.tensor.transpose(tp[:], nt[:], ident[:])
            nc.vector.tensor_copy(
                attnT[r0 : r0 + DH, qb * P : (qb + 1) * P], tp[:]
            )

    def yproj_pair(m, first):
        # fold this head-pair's slice of the output projection into y_acc
        attnT = attnT_cell[m]
        for qb in range(QB):
            pt = po_proj.tile([P, E], F32, tag="pop", name=f"py{m}_{qb}")
            nc.tensor.matmul(
                pt[:],
                lhsT=attnT[:, qb * P : (qb + 1) * P],
                rhs=w16["Wo"][:, m, :],
                start=True,
                stop=True,
            )
            ya = y_acc_cell[0]
            if first:
                nc.vector.tensor_add(ya[:, qb, :], pt[:], bo_b[:])
            else:
                nc.vector.tensor_add(ya[:, qb, :], ya[:, qb, :], pt[:])

    # --- emission order: overlap loads/projections with attention --------
    # (Tile derives dependencies from program order, so every consumer is
    # emitted after its producer; the scheduler then overlaps freely.)
    load_x_quarter(0)
    load_w("Wk")
    proj_q(0)
    proj_k(0, 0, 2)
    load_w("Wv")
    bv_b = bcast("bv")
    bo_b = bcast("bo")
    proj_v(0, 8)
    for m in range(1, NM):
        proj_q(m)
        proj_k(m, 0, 2)
    for _m in range(NM):
        attnT_cell.append(big.tile([P, SQ], F16, tag=f"attnT{_m}", name=f"attnT{_m}"))
    y_acc = big.tile([P, QB, E], F32, tag="xTq", name="y_acc")
    y_acc_cell.append(y_acc)
    load_x_quarter(1)
    for sw in range(NSW):
        qtr = sw // 2
        if sw > 0 and sw % 2 == 0:
            for m in range(NM):
                proj_k(m, 2 * qtr, 2 * qtr + 2)
            proj_v(8 * qtr, 8 * qtr + 8)
        for m in range(NM):
            pair_sweep(m, sw)
            if sw == 0 and m == 0:
                load_w("Wo")
            if sw % 2 == 0 and m == 2 and qtr + 2 <= 3:
                load_x_quarter(qtr + 2)
            if sw == NSW - 1:
                finish_head(2 * m)
                finish_head(2 * m + 1)
                yproj_pair(m, first=(m == 0))

    # --- output writeback ------------------------------------------------
    for qb in range(QB):
        eng = nc.sync if qb % 2 == 0 else nc.gpsimd
        eng.dma_start(y[qb * P : (qb + 1) * P, :], y_acc_cell[0][:, qb, :])


def build():
    nc = bacc.Bacc("TRN2", target_bir_lowering=False, debug=False)
    io = {}
    io["xT_kv"] = nc.dram_tensor("xT_kv", [E, S], F16, kind="ExternalInput").ap()
    io["xT_q"] = nc.dram_tensor("xT_q", [E, SQ], F16, kind="ExternalInput").ap()
    for n in ("Wq", "Wk", "Wv", "Wo"):
        io[n] = nc.dram_tensor(n, [E, E], F16, kind="ExternalInput").ap()
    for n in ("bq", "bk", "bv", "bo"):
        io[n] = nc.dram_tensor(n, [E], F32, kind="ExternalInput").ap()
    io["y"] = nc.dram_tensor("y", [SQ, E], F32, kind="ExternalOutput").ap()
    with tile.TileContext(nc) as tc:
        with ExitStack() as ctx:
            emit(ctx, tc, io)
    nc.compile()
    return nc


_NC = None


def _get_nc():
    global _NC
    if _NC is None:
        _NC = build()
    return _NC


def shard_inputs(x, Wq, bq, Wk, bk, Wv, bv, Wo, bo):
    x16 = x.astype(np.float16)
    W16 = [w.astype(np.float16) for w in (Wq, Wk, Wv, Wo)]
    maps = []
    for c in range(8):
        b, qi = c // 4, c % 4
        maps.append(
            {
                "xT_kv": np.ascontiguousarray(x16[b].T),
                "xT_q": np.ascontiguousarray(x16[b, qi * SQ : (qi + 1) * SQ].T),
                "Wq": W16[0], "Wk": W16[1], "Wv": W16[2], "Wo": W16[3],
                "bq": bq, "bk": bk, "bv": bv, "bo": bo,
            }
        )
    return maps


def kernel(x, Wq, bq, Wk, bk, Wv, bv, Wo, bo):
    args = [np.ascontiguousarray(np.asarray(a, dtype=np.float32))
            for a in (x, Wq, bq, Wk, bk, Wv, bv, Wo, bo)]
    nc = _get_nc()
    maps = shard_inputs(*args)
    res = run_bass_kernel_spmd(nc, maps, list(range(8)))
    out = np.empty((B, S, E), dtype=np.float32)
    for c in range(8):
        b, qi = c // 4, c % 4
        out[b, qi * SQ : (qi + 1) * SQ] = res.results[c]["y"]
    return out


# revision 41
# speedup vs baseline: 1.0383x; 1.0383x over previous
"""Multi-head attention (B=2, S=4096, E=512, H=8) on 8 Trainium2 NeuronCores.

Sharding: core c handles batch b = c//4 and query rows [qi*1024, (qi+1)*1024)
with qi = c%4. Each core computes K/V projections for its full batch (cheap,
avoids any cross-core reduction), Q projection for its query slice, attention
for all 8 heads over its query rows, and the output projection for its rows.
The host only transposes shards on the way in and concatenates the 8 per-core
[1024, 512] outputs on the way out.

Layout/precision choices (from HW traces):
- All matmul operands are fp16 (same PE rate as bf16, 8x the mantissa);
  x and the weight matrices are rounded to fp16 on the host so the device
  DMAs half the bytes and runs no cast pass. All accumulation is fp32.
- Scores are computed transposed (S^T[k, q]) so the exp'd weights land in
  the [k, q] layout the AV matmul needs as its stationary operand; softmax
  denominators come free from an all-ones column appended to V.
- Q^T is stored zero-padded to 128 partitions per head: a 64-partition
  moving operand streams at half rate on the PE, and the zero half lets the
  two heads of a 128-row m-tile share one stationary K^T load.
- Max-subtraction is skipped: raw scores stay well inside fp32/fp16 range
  here, and softmax without the shift is mathematically identical.
- The ACT engine (exp over 33.5M scores/core, ~1.05us per [128,1024] tile)
  is the bottleneck; emission order keeps it fed from ~35us onward.
"""

from contextlib import ExitStack

import numpy as np

import concourse.bass as bass
import concourse.tile as tile
from concourse import bacc, masks, mybir
from concourse.bass_utils import run_bass_kernel_spmd

F32 = mybir.dt.float32
F16 = mybir.dt.float16
AF = mybir.ActivationFunctionType

B = 2
S = 4096  # keys per batch
SQ = 1024  # query rows per core
E = 512
H = 8
DH = 64
P = 128
NE = E // P  # 4 chunks of the contraction dim E
NM = E // P  # 4 m-tiles (head-pairs)
KB = S // P  # 32 key blocks
QB = SQ // P  # 8 query blocks
SC = S // 512  # 8 key 512-chunks
QC = SQ // 512  # 2 query 512-chunks
SWL = 4  # key blocks per attention sweep
NSW = KB // SWL
SCALE = 1.0 / np.sqrt(DH)


def emit(ctx: ExitStack, tc: tile.TileContext, io: dict):
    nc = tc.nc
    xT = io["xT_kv"]  # [E, S] f32
    xTq = io["xT_q"]  # [E, SQ] f32
    y = io["y"]  # [SQ, E] f32

    const = ctx.enter_context(tc.tile_pool(name="const", bufs=1))
    big = ctx.enter_context(tc.tile_pool(name="big", bufs=1))
    apool = ctx.enter_context(tc.tile_pool(name="apool", bufs=2))
    npool = ctx.enter_context(tc.tile_pool(name="npool", bufs=2))
    rpool = ctx.enter_context(tc.tile_pool(name="rpool", bufs=2))
    ypool = ctx.enter_context(tc.tile_pool(name="ypool", bufs=2))
    opool = ctx.enter_context(tc.tile_pool(name="opool", bufs=H))
    # PSUM: 8 banks. Scores rotate through a 6-bank fp16 tensor in aligned
    # (even, odd) head-pairs so exp reads [128, 2048]; projections, AV
    # accumulators, transposes and the output projection share two rotating
    # 1-bank slots (tag "po").
    ps_att = ctx.enter_context(tc.tile_pool(name="ps_att", bufs=2, space="PSUM"))
    po = ctx.enter_context(tc.tile_pool(name="po", bufs=2, space="PSUM"))
    po_proj = ctx.enter_context(tc.tile_pool(name="po_proj", bufs=2, space="PSUM"))

    # --- constants -------------------------------------------------------
    ident = const.tile([P, P], F16)
    masks.make_identity(nc, ident[:])

    w16 = {}

    def load_w(name):
        wt = const.tile([P, NE, E], F16, tag=f"w_{name}", name=f"w_{name}")
        for ec in range(NE):
            eng = nc.sync if ec % 2 == 0 else nc.gpsimd
            eng.dma_start(wt[:, ec, :], io[name][ec * P : (ec + 1) * P, :])
        w16[name] = wt

    load_w("Wq")

    bq_t = const.tile([P, NM], F32, tag="bq")
    bk_t = const.tile([P, NM], F32, tag="bk")
    for m in range(NM):
        nc.gpsimd.dma_start(bq_t[:, m : m + 1], io["bq"][m * P : (m + 1) * P])
        nc.gpsimd.dma_start(bk_t[:, m : m + 1], io["bk"][m * P : (m + 1) * P])

    def bcast(name):
        t = const.tile([P, E], F32, tag=f"bcast_{name}", name=f"bc_{name}")
        src = io[name]
        ap = bass.AP(tensor=src.tensor, offset=src.offset, ap=[[0, P]] + list(src.ap))
        nc.gpsimd.dma_start(t[:], ap)
        return t

    # --- x^T loads (quarter-major so projections can start early) --------
    xT16 = big.tile([P, NE, S], F16, tag="xT")
    xTq16 = big.tile([P, NE, SQ], F16, tag="xTq")
    XQ = 1024  # x columns per load quarter

    for ec in range(NE):
        eng = nc.sync if ec % 2 == 0 else nc.gpsimd
        eng.dma_start(xTq16[:, ec, :], xTq[ec * P : (ec + 1) * P, :])

    def load_x_quarter(qtr):
        for ec in range(NE):
            eng = nc.sync if ec % 2 == 0 else nc.gpsimd
            eng.dma_start(
                xT16[:, ec, qtr * XQ : (qtr + 1) * XQ],
                xT[ec * P : (ec + 1) * P, qtr * XQ : (qtr + 1) * XQ],
            )

    # --- projection targets ----------------------------------------------
    kT16 = big.tile([P, NM, S], F16, tag="kT")
    # Q^T padded: head h lives in [:, h, :] rows (h%2)*64..+64, rest zero.
    qTp = big.tile([P, H, SQ], F16, tag="qTp")
    nc.gpsimd.memset(qTp[:], 0.0)
    v16 = big.tile([P, KB, H, DH + 1], F16, tag="v")
    nc.gpsimd.memset(v16[:, :, :, DH : DH + 1], 1.0)

    def proj_k(m, sc0, sc1):
        for sc in range(sc0, sc1):
            pt = po_proj.tile([P, 512], F32, tag="pop", name=f"pk{m}_{sc}")
            for ec in range(NE):
                nc.tensor.matmul(
                    pt[:],
                    lhsT=w16["Wk"][:, ec, m * P : (m + 1) * P],
                    rhs=xT16[:, ec, sc * 512 : (sc + 1) * 512],
                    start=(ec == 0),
                    stop=(ec == NE - 1),
                )
            nc.vector.tensor_scalar_add(
                kT16[:, m, sc * 512 : (sc + 1) * 512], pt[:], bk_t[:, m : m + 1]
            )

    def proj_q(m):
        for qc in range(QC):
            pt = po_proj.tile([P, 512], F32, tag="pop", name=f"pq{m}_{qc}")
            for ec in range(NE):
                nc.tensor.matmul(
                    pt[:],
                    lhsT=w16["Wq"][:, ec, m * P : (m + 1) * P],
                    rhs=xTq16[:, ec, qc * 512 : (qc + 1) * 512],
                    start=(ec == 0),
                    stop=(ec == NE - 1),
                )
            for hh in range(2):
                h = 2 * m + hh
                r0 = hh * DH
                nc.vector.tensor_scalar_add(
                    qTp[r0 : r0 + DH, h, qc * 512 : (qc + 1) * 512],
                    pt[r0 : r0 + DH, :],
                    bq_t[r0 : r0 + DH, m : m + 1],
                )

    def proj_v(sb0, sb1):
        for sb in range(sb0, sb1):
            pt = po_proj.tile([P, 512], F32, tag="pop", name=f"pv{sb}")
            for ec in range(NE):
                nc.tensor.matmul(
                    pt[:],
                    lhsT=xT16[:, ec, sb * P : (sb + 1) * P],
                    rhs=w16["Wv"][:, ec, :],
                    start=(ec == 0),
                    stop=(ec == NE - 1),
                )
            nc.vector.tensor_add(
                v16[:, sb, :, 0:DH],
                pt[:].rearrange("p (h d) -> p h d", h=H),
                bv_b[:].rearrange("p (h d) -> p h d", h=H),
            )

    # --- attention --------------------------------------------------------

    # attnT reuses xTq16's slot (tag "xTq"): allocated after proj_q's last
    # read of xTq16, by which point the slot is free.
    attnT_cell = []
    o_accs = {}
    rot = [0]  # scores_ps slot rotation

    def pair_sweep(m, sw):
        """Scores+exp+AV for head pair (2m, 2m+1), key blocks sw*SWL..+SWL."""
        heads = (2 * m, 2 * m + 1)
        if sw == 0:
            for h in heads:
                o_accs[h] = opool.tile(
                    [P, QB, DH + 1], F32, tag="oacc", name=f"oacc{h}"
                )
        at = apool.tile([P, SWL, 2, SQ], F16, tag="a")
        for i in range(SWL):
            kb = sw * SWL + i
            # one stationary K^T load serves both heads; the zero half of
            # each head's padded Q^T kills the other head's contribution
            for hh in range(2):
                st_ = ps_att.tile([P, SQ], F32, tag="psa", name=f"sc{m}_{sw}_{i}{hh}")
                for qc in range(QC):
                    nc.tensor.matmul(
                        st_[:, qc * 512 : (qc + 1) * 512],
                        lhsT=kT16[:, m, kb * P : (kb + 1) * P],
                        rhs=qTp[:, heads[hh], qc * 512 : (qc + 1) * 512],
                        start=True,
                        stop=True,
                    )
                nc.scalar.activation(
                    at[:, i, hh, :], st_[:], AF.Exp, scale=float(SCALE)
                )
        for hh in range(2):
            h = heads[hh]
            o_acc = o_accs[h]
            for qb in range(QB):
                ot = po.tile([P, DH + 1], F32, tag="po", name=f"ot{h}_{qb}")
                for i in range(SWL):
                    kb = sw * SWL + i
                    nc.tensor.matmul(
                        ot[:],
                        lhsT=at[:, i, hh, qb * P : (qb + 1) * P],
                        rhs=v16[:, kb, h, :],
                        start=(i == 0),
                        stop=(i == SWL - 1),
                    )
                if sw == 0:
                    nc.vector.tensor_copy(o_acc[:, qb, :], ot[:])
                else:
                    nc.vector.tensor_add(o_acc[:, qb, :], o_acc[:, qb, :], ot[:])

    def finish_head(h):
        m, r0 = h // 2, (h % 2) * DH
        o_acc = o_accs[h]
        attnT = attnT_cell[0]
        for qb in range(QB):
            rt = rpool.tile([P, 1], F32, tag="r")
            nc.vector.reciprocal(rt[:], o_acc[:, qb, DH : DH + 1])
            nt = npool.tile([P, DH], F16, tag="n")
            nc.vector.tensor_scalar_mul(nt[:], o_acc[:, qb, 0:DH], rt[:])
            tp = po.tile([DH, P], F16, tag="po", name=f"tp{h}_{qb}")
            nc.tensor.transpose(tp[:], nt[:], ident[:])
            nc.vector.tensor_copy(
                attnT[r0 : r0 + DH, m, qb * P : (qb + 1) * P], tp[:]
            )

    # --- emission order: overlap loads/projections with attention --------
    # (Tile derives dependencies from program order, so every consumer is
    # emitted after its producer; the scheduler then overlaps freely.)
    load_x_quarter(0)
    load_w("Wk")
    proj_q(0)
    proj_k(0, 0, 2)
    load_w("Wv")
    bv_b = bcast("bv")
    bo_b = bcast("bo")
    proj_v(0, 8)
    for m in range(1, NM):
        proj_q(m)
        proj_k(m, 0, 2)
    attnT = big.tile([P, NE, SQ], F16, tag="xTq", name="attnT")
    attnT_cell.append(attnT)
    load_x_quarter(1)
    for sw in range(NSW):
        qtr = sw // 2
        if sw > 0 and sw % 2 == 0:
            for m in range(NM):
                proj_k(m, 2 * qtr, 2 * qtr + 2)
            proj_v(8 * qtr, 8 * qtr + 8)
        for m in range(NM):
            pair_sweep(m, sw)
            if sw == 0 and m == 0:
                load_w("Wo")
            if sw % 2 == 0 and m == 2 and qtr + 2 <= 3:
                load_x_quarter(qtr + 2)
            if sw == NSW - 1:
                finish_head(2 * m)
                finish_head(2 * m + 1)

    # --- output projection ----------------------------------------------
    attnT = attnT_cell[0]
    for qb in range(QB):
        pt = po_proj.tile([P, E], F32, tag="pop", name=f"py{qb}")
        for c in range(NE):
            nc.tensor.matmul(
                pt[:],
                lhsT=attnT[:, c, qb * P : (qb + 1) * P],
                rhs=w16["Wo"][:, c, :],
                start=(c == 0),
                stop=(c == NE - 1),
            )
        yt = ypool.tile([P, E], F32, tag="y")
        nc.vector.tensor_add(yt[:], pt[:], bo_b[:])
        nc.sync.dma_start(y[qb * P : (qb + 1) * P, :], yt[:])


def build():
    nc = bacc.Bacc("TRN2", target_bir_lowering=False, debug=False)
    io = {}
    io["xT_kv"] = nc.dram_tensor("xT_kv", [E, S], F16, kind="ExternalInput").ap()
    io["xT_q"] = nc.dram_tensor("xT_q", [E, SQ], F16, kind="ExternalInput").ap()
    for n in ("Wq", "Wk", "Wv", "Wo"):
        io[n] = nc.dram_tensor(n, [E, E], F16, kind="ExternalInput").ap()
    for n in ("bq", "bk", "bv", "bo"):
        io[n] = nc.dram_tensor(n, [E], F32, kind="ExternalInput").ap()
    io["y"] = nc.dram_tensor("y", [SQ, E], F32, kind="ExternalOutput").ap()
    with tile.TileContext(nc) as tc:
        with ExitStack() as ctx:
            emit(ctx, tc, io)
    nc.compile()
    return nc


_NC = None


def _get_nc():
    global _NC
    if _NC is None:
        _NC = build()
    return _NC


def shard_inputs(x, Wq, bq, Wk, bk, Wv, bv, Wo, bo):
    x16 = x.astype(np.float16)
    W16 = [w.astype(np.float16) for w in (Wq, Wk, Wv, Wo)]
    maps = []
    for c in range(8):
        b, qi = c // 4, c % 4
        maps.append(
            {
                "xT_kv": np.ascontiguousarray(x16[b].T),
                "xT_q": np.ascontiguousarray(x16[b, qi * SQ : (qi + 1) * SQ].T),
                "Wq": W16[0], "Wk": W16[1], "Wv": W16[2], "Wo": W16[3],
                "bq": bq, "bk": bk, "bv": bv, "bo": bo,
            }
        )
    return maps


def kernel(x, Wq, bq, Wk, bk, Wv, bv, Wo, bo):
    args = [np.ascontiguousarray(np.asarray(a, dtype=np.float32))
            for a in (x, Wq, bq, Wk, bk, Wv, bv, Wo, bo)]
    nc = _get_nc()
    maps = shard_inputs(*args)
    res = run_bass_kernel_spmd(nc, maps, list(range(8)))
    out = np.empty((B, S, E), dtype=np.float32)
    for c in range(8):
        b, qi = c // 4, c % 4
        out[b, qi * SQ : (qi + 1) * SQ] = res.results[c]["y"]
    return out


# revision 42
# speedup vs baseline: 1.0546x; 1.0157x over previous
"""Multi-head attention (B=2, S=4096, E=512, H=8) on 8 Trainium2 NeuronCores.

Sharding: core c handles batch b = c//4 and query rows [qi*1024, (qi+1)*1024)
with qi = c%4. Each core computes K/V projections for its full batch (cheap,
avoids any cross-core reduction), Q projection for its query slice, attention
for all 8 heads over its query rows, and the output projection for its rows.
The host only transposes shards on the way in and concatenates the 8 per-core
[1024, 512] outputs on the way out.

Layout/precision choices (from HW traces):
- All matmul operands are fp16 (same PE rate as bf16, 8x the mantissa);
  x and the weight matrices are rounded to fp16 on the host so the device
  DMAs half the bytes and runs no cast pass. All accumulation is fp32.
- Scores are computed transposed (S^T[k, q]) so the exp'd weights land in
  the [k, q] layout the AV matmul needs as its stationary operand; softmax
  denominators come free from an all-ones column appended to V.
- Q^T is stored zero-padded to 128 partitions per head: a 64-partition
  moving operand streams at half rate on the PE, and the zero half lets the
  two heads of a 128-row m-tile share one stationary K^T load.
- Max-subtraction is skipped: raw scores stay well inside fp32/fp16 range
  here, and softmax without the shift is mathematically identical.
- The ACT engine (exp over 33.5M scores/core, ~1.05us per [128,1024] tile)
  is the bottleneck; emission order keeps it fed from ~35us onward.
"""

from contextlib import ExitStack

import numpy as np

import concourse.bass as bass
import concourse.tile as tile
from concourse import bacc, masks, mybir
from concourse.bass_utils import run_bass_kernel_spmd

F32 = mybir.dt.float32
F16 = mybir.dt.float16
AF = mybir.ActivationFunctionType

B = 2
S = 4096  # keys per batch
SQ = 1024  # query rows per core
E = 512
H = 8
DH = 64
P = 128
NE = E // P  # 4 chunks of the contraction dim E
NM = E // P  # 4 m-tiles (head-pairs)
KB = S // P  # 32 key blocks
QB = SQ // P  # 8 query blocks
SC = S // 512  # 8 key 512-chunks
QC = SQ // 512  # 2 query 512-chunks
SWL = 4  # key blocks per attention sweep
NSW = KB // SWL
SCALE = 1.0 / np.sqrt(DH)


def emit(ctx: ExitStack, tc: tile.TileContext, io: dict):
    nc = tc.nc
    xT = io["xT_kv"]  # [E, S] f32
    xTq = io["xT_q"]  # [E, SQ] f32
    y = io["y"]  # [SQ, E] f32

    const = ctx.enter_context(tc.tile_pool(name="const", bufs=1))
    big = ctx.enter_context(tc.tile_pool(name="big", bufs=1))
    apool = ctx.enter_context(tc.tile_pool(name="apool", bufs=2))
    npool = ctx.enter_context(tc.tile_pool(name="npool", bufs=2))
    rpool = ctx.enter_context(tc.tile_pool(name="rpool", bufs=2))
    ypool = ctx.enter_context(tc.tile_pool(name="ypool", bufs=2))
    opool = ctx.enter_context(tc.tile_pool(name="opool", bufs=H))
    # PSUM: 8 banks. Scores rotate through a 6-bank fp16 tensor in aligned
    # (even, odd) head-pairs so exp reads [128, 2048]; projections, AV
    # accumulators, transposes and the output projection share two rotating
    # 1-bank slots (tag "po").
    ps_att = ctx.enter_context(tc.tile_pool(name="ps_att", bufs=2, space="PSUM"))
    po = ctx.enter_context(tc.tile_pool(name="po", bufs=2, space="PSUM"))
    po_proj = ctx.enter_context(tc.tile_pool(name="po_proj", bufs=2, space="PSUM"))

    # --- constants -------------------------------------------------------
    ident = const.tile([P, P], F16)
    masks.make_identity(nc, ident[:])

    w16 = {}

    def load_w(name):
        wt = const.tile([P, NE, E], F16, tag=f"w_{name}", name=f"w_{name}")
        for ec in range(NE):
            eng = nc.sync if ec % 2 == 0 else nc.gpsimd
            eng.dma_start(wt[:, ec, :], io[name][ec * P : (ec + 1) * P, :])
        w16[name] = wt

    load_w("Wq")

    bq_t = const.tile([P, NM], F32, tag="bq")
    bk_t = const.tile([P, NM], F32, tag="bk")
    for m in range(NM):
        nc.gpsimd.dma_start(bq_t[:, m : m + 1], io["bq"][m * P : (m + 1) * P])
        nc.gpsimd.dma_start(bk_t[:, m : m + 1], io["bk"][m * P : (m + 1) * P])

    def bcast(name):
        t = const.tile([P, E], F32, tag=f"bcast_{name}", name=f"bc_{name}")
        src = io[name]
        ap = bass.AP(tensor=src.tensor, offset=src.offset, ap=[[0, P]] + list(src.ap))
        nc.gpsimd.dma_start(t[:], ap)
        return t

    # --- x^T loads (quarter-major so projections can start early) --------
    xT16 = big.tile([P, NE, S], F16, tag="xT")
    xTq16 = big.tile([P, NE, SQ], F16, tag="xTq")
    XQ = 1024  # x columns per load quarter

    for ec in range(NE):
        nc.sync.dma_start(xTq16[:, ec, :], xTq[ec * P : (ec + 1) * P, :])

    def load_x_quarter(qtr):
        for ec in range(NE):
            eng = nc.sync if (qtr == 0 or ec % 2 == 0) else nc.gpsimd
            eng.dma_start(
                xT16[:, ec, qtr * XQ : (qtr + 1) * XQ],
                xT[ec * P : (ec + 1) * P, qtr * XQ : (qtr + 1) * XQ],
            )

    # --- projection targets ----------------------------------------------
    kT16 = big.tile([P, NM, S], F16, tag="kT")
    # Q^T padded: head h lives in [:, h, :] rows (h%2)*64..+64, rest zero.
    qTp = big.tile([P, H, SQ], F16, tag="qTp")
    nc.vector.memset(qTp[:, 0:4, :], 0.0)
    nc.vector.memset(qTp[:, 4:8, :], 0.0)
    v16 = big.tile([P, KB, H, DH + 1], F16, tag="v")
    nc.vector.memset(v16[:, :, :, DH : DH + 1], 1.0)

    def proj_k(m, sc0, sc1):
        for sc in range(sc0, sc1):
            pt = po_proj.tile([P, 512], F32, tag="pop", name=f"pk{m}_{sc}")
            for ec in range(NE):
                nc.tensor.matmul(
                    pt[:],
                    lhsT=w16["Wk"][:, ec, m * P : (m + 1) * P],
                    rhs=xT16[:, ec, sc * 512 : (sc + 1) * 512],
                    start=(ec == 0),
                    stop=(ec == NE - 1),
                )
            nc.vector.tensor_scalar_add(
                kT16[:, m, sc * 512 : (sc + 1) * 512], pt[:], bk_t[:, m : m + 1]
            )

    def proj_q(m):
        for qc in range(QC):
            pt = po_proj.tile([P, 512], F32, tag="pop", name=f"pq{m}_{qc}")
            for ec in range(NE):
                nc.tensor.matmul(
                    pt[:],
                    lhsT=w16["Wq"][:, ec, m * P : (m + 1) * P],
                    rhs=xTq16[:, ec, qc * 512 : (qc + 1) * 512],
                    start=(ec == 0),
                    stop=(ec == NE - 1),
                )
            for hh in range(2):
                h = 2 * m + hh
                r0 = hh * DH
                nc.vector.tensor_scalar_add(
                    qTp[r0 : r0 + DH, h, qc * 512 : (qc + 1) * 512],
                    pt[r0 : r0 + DH, :],
                    bq_t[r0 : r0 + DH, m : m + 1],
                )

    def proj_v(sb0, sb1):
        for sb in range(sb0, sb1):
            pt = po_proj.tile([P, 512], F32, tag="pop", name=f"pv{sb}")
            for ec in range(NE):
                nc.tensor.matmul(
                    pt[:],
                    lhsT=xT16[:, ec, sb * P : (sb + 1) * P],
                    rhs=w16["Wv"][:, ec, :],
                    start=(ec == 0),
                    stop=(ec == NE - 1),
                )
            nc.vector.tensor_add(
                v16[:, sb, :, 0:DH],
                pt[:].rearrange("p (h d) -> p h d", h=H),
                bv_b[:].rearrange("p (h d) -> p h d", h=H),
            )

    # --- attention --------------------------------------------------------

    # attnT reuses xTq16's slot (tag "xTq"): allocated after proj_q's last
    # read of xTq16, by which point the slot is free.
    attnT_cell = []
    o_accs = {}
    rot = [0]  # scores_ps slot rotation

    def pair_sweep(m, sw):
        """Scores+exp+AV for head pair (2m, 2m+1), key blocks sw*SWL..+SWL."""
        heads = (2 * m, 2 * m + 1)
        if sw == 0:
            for h in heads:
                o_accs[h] = opool.tile(
                    [P, QB, DH + 1], F32, tag="oacc", name=f"oacc{h}"
                )
        at = apool.tile([P, SWL, 2, SQ], F16, tag="a")
        for i in range(SWL):
            kb = sw * SWL + i
            # one stationary K^T load serves both heads; the zero half of
            # each head's padded Q^T kills the other head's contribution
            for hh in range(2):
                st_ = ps_att.tile([P, SQ], F32, tag="psa", name=f"sc{m}_{sw}_{i}{hh}")
                for qc in range(QC):
                    nc.tensor.matmul(
                        st_[:, qc * 512 : (qc + 1) * 512],
                        lhsT=kT16[:, m, kb * P : (kb + 1) * P],
                        rhs=qTp[:, heads[hh], qc * 512 : (qc + 1) * 512],
                        start=True,
                        stop=True,
                    )
                nc.scalar.activation(
                    at[:, i, hh, :], st_[:], AF.Exp, scale=float(SCALE)
                )
        for hh in range(2):
            h = heads[hh]
            o_acc = o_accs[h]
            for qb in range(QB):
                ot = po.tile([P, DH + 1], F32, tag="po", name=f"ot{h}_{qb}")
                for i in range(SWL):
                    kb = sw * SWL + i
                    nc.tensor.matmul(
                        ot[:],
                        lhsT=at[:, i, hh, qb * P : (qb + 1) * P],
                        rhs=v16[:, kb, h, :],
                        start=(i == 0),
                        stop=(i == SWL - 1),
                    )
                if sw == 0:
                    nc.vector.tensor_copy(o_acc[:, qb, :], ot[:])
                else:
                    nc.vector.tensor_add(o_acc[:, qb, :], o_acc[:, qb, :], ot[:])

    def finish_head(h):
        m, r0 = h // 2, (h % 2) * DH
        o_acc = o_accs[h]
        attnT = attnT_cell[0]
        for qb in range(QB):
            rt = rpool.tile([P, 1], F32, tag="r")
            nc.vector.reciprocal(rt[:], o_acc[:, qb, DH : DH + 1])
            nt = npool.tile([P, DH], F16, tag="n")
            nc.vector.tensor_scalar_mul(nt[:], o_acc[:, qb, 0:DH], rt[:])
            tp = po.tile([DH, P], F16, tag="po", name=f"tp{h}_{qb}")
            nc.tensor.transpose(tp[:], nt[:], ident[:])
            nc.vector.tensor_copy(
                attnT[r0 : r0 + DH, m, qb * P : (qb + 1) * P], tp[:]
            )

    # --- emission order: overlap loads/projections with attention --------
    # (Tile derives dependencies from program order, so every consumer is
    # emitted after its producer; the scheduler then overlaps freely.)
    load_x_quarter(0)
    load_w("Wk")
    proj_q(0)
    proj_k(0, 0, 2)
    load_w("Wv")
    bv_b = bcast("bv")
    bo_b = bcast("bo")
    proj_v(0, 8)
    for m in range(1, NM):
        proj_q(m)
        proj_k(m, 0, 2)
    attnT = big.tile([P, NE, SQ], F16, tag="xTq", name="attnT")
    attnT_cell.append(attnT)
    load_x_quarter(1)
    for sw in range(NSW):
        qtr = sw // 2
        if sw > 0 and sw % 2 == 0:
            for m in range(NM):
                proj_k(m, 2 * qtr, 2 * qtr + 2)
            proj_v(8 * qtr, 8 * qtr + 8)
        for m in range(NM):
            pair_sweep(m, sw)
            if sw == 0 and m == 0:
                load_w("Wo")
            if sw % 2 == 0 and m == 2 and qtr + 2 <= 3:
                load_x_quarter(qtr + 2)
            if sw == NSW - 1:
                finish_head(2 * m)
                finish_head(2 * m + 1)

    # --- output projection ----------------------------------------------
    attnT = attnT_cell[0]
    for qb in range(QB):
        pt = po_proj.tile([P, E], F32, tag="pop", name=f"py{qb}")
        for c in range(NE):
            nc.tensor.matmul(
                pt[:],
                lhsT=attnT[:, c, qb * P : (qb + 1) * P],
                rhs=w16["Wo"][:, c, :],
                start=(c == 0),
                stop=(c == NE - 1),
            )
        yt = ypool.tile([P, E], F32, tag="y")
        nc.vector.tensor_add(yt[:], pt[:], bo_b[:])
        eng = nc.sync if qb % 2 == 0 else nc.gpsimd
        eng.dma_start(y[qb * P : (qb + 1) * P, :], yt[:])


def build():
    nc = bacc.Bacc("TRN2", target_bir_lowering=False, debug=False)
    io = {}
    io["xT_kv"] = nc.dram_tensor("xT_kv", [E, S], F16, kind="ExternalInput").ap()
    io["xT_q"] = nc.dram_tensor("xT_q", [E, SQ], F16, kind="ExternalInput").ap()
    for n in ("Wq", "Wk", "Wv", "Wo"):
        io[n] = nc.dram_tensor(n, [E, E], F16, kind="ExternalInput").ap()
    for n in ("bq", "bk", "bv", "bo"):
        io[n] = nc.dram_tensor(n, [E], F32, kind="ExternalInput").ap()
    io["y"] = nc.dram_tensor("y", [SQ, E], F32, kind="ExternalOutput").ap()
    with tile.TileContext(nc) as tc:
        with ExitStack() as ctx:
            emit(ctx, tc, io)
    nc.compile()
    return nc


_NC = None


def _get_nc():
    global _NC
    if _NC is None:
        _NC = build()
    return _NC


def shard_inputs(x, Wq, bq, Wk, bk, Wv, bv, Wo, bo):
    x16 = x.astype(np.float16)
    W16 = [w.astype(np.float16) for w in (Wq, Wk, Wv, Wo)]
    maps = []
    for c in range(8):
        b, qi = c // 4, c % 4
        maps.append(
            {
                "xT_kv": np.ascontiguousarray(x16[b].T),
                "xT_q": np.ascontiguousarray(x16[b, qi * SQ : (qi + 1) * SQ].T),
                "Wq": W16[0], "Wk": W16[1], "Wv": W16[2], "Wo": W16[3],
                "bq": bq, "bk": bk, "bv": bv, "bo": bo,
            }
        )
    return maps


def kernel(x, Wq, bq, Wk, bk, Wv, bv, Wo, bo):
    args = [np.ascontiguousarray(np.asarray(a, dtype=np.float32))
            for a in (x, Wq, bq, Wk, bk, Wv, bv, Wo, bo)]
    nc = _get_nc()
    maps = shard_inputs(*args)
    res = run_bass_kernel_spmd(nc, maps, list(range(8)))
    out = np.empty((B, S, E), dtype=np.float32)
    for c in range(8):
        b, qi = c // 4, c % 4
        out[b, qi * SQ : (qi + 1) * SQ] = res.results[c]["y"]
    return out
